# revision 25
# baseline (speedup 1.0000x reference)
"""VQ codebook lookup (CategoricalEmbedding.straight_through) on 8 TRN2 cores.

Data-parallel: core c takes batch row c of z_e_x [8, 2048, 512]; the
[4096, 512] codebook is replicated. Per core:
    nd[n, k] = S[n, k] - fl(cb_sqr'[k] + x_sqr'[n])      (= -4096 * distance)
    idx[n]   = argmax_k nd[n, k]   (first index on ties == jnp.argmin of d)
    z[n]     = codebook[idx[n]]

Bit-exactness vs the jax reference expression
    (cb_sqr + x_sqr) - 2.0 * (x @ c^T)
is preserved by computing S = (2x) @ (4096*c)^T with an fp16 hi/lo 3-pass
matmul (error ~6e-9, i.e. fp32-grade; fp16 products are exact in fp32 and
accumulate in fp32 PSUM) and scaling cb_sqr/x_sqr by 4096 host-side:
power-of-two scaling commutes exactly with fp rounding, so the scaled
subtract rounds identically and argmax/ties are unchanged.
"""

import os
from contextlib import ExitStack

import numpy as np

import concourse.bass as bass
import concourse.mybir as mybir
import concourse.tile as tile
from concourse import bacc
from concourse.bass_utils import run_bass_kernel_spmd

P = 128

# Problem geometry (hardcoded per contract)
B, S, D = 8, 2048, 512
K = 4096
N_CORES = 8

MM_MODE = os.environ.get("VQ_MM_MODE", "f16_3p")  # "f16_3p" | "f32"


def build_bass(n=S, k=K, d=D, kc_width=512, mode=None, offload=True,
               use_reduce_max=False, psum_bufs=4):
    """Build the per-core Bass program. All cores run the same program.

    offload: route psum->sbuf copies to ACT and the subtracts to GPSIMD,
    leaving DVE only the max/max_index scans.
    """
    mode = mode or MM_MODE
    assert n % P == 0 and d % P == 0 and k % kc_width == 0
    nt = n // P   # row tiles
    ndt = d // P  # contraction tiles
    nkc = k // kc_width  # codebook chunks

    nc = bacc.Bacc("TRN2", target_bir_lowering=False)
    f32 = mybir.dt.float32
    f16 = mybir.dt.float16

    if mode == "f32":
        x2t = nc.declare_dram_parameter("x2t", [d, n], f32, isOutput=False)
        ct = nc.declare_dram_parameter("ct", [d, k], f32, isOutput=False)
    else:
        x2t_hi = nc.declare_dram_parameter("x2t_hi", [d, n], f16, isOutput=False)
        x2t_lo = nc.declare_dram_parameter("x2t_lo", [d, n], f16, isOutput=False)
        ct_hi = nc.declare_dram_parameter("ct_hi", [d, k], f16, isOutput=False)
        ct_lo = nc.declare_dram_parameter("ct_lo", [d, k], f16, isOutput=False)
    cb = nc.declare_dram_parameter("cb", [k, d], f32, isOutput=False)
    cbs_rep = nc.declare_dram_parameter("cbs_rep", [P, k], f32, isOutput=False)
    xsq = nc.declare_dram_parameter("xsq", [nt, P], f32, isOutput=False)
    z_out = nc.declare_dram_parameter("z_out", [n, d], f32, isOutput=True)
    idx_out = nc.declare_dram_parameter("idx_out", [n, 1], mybir.dt.uint32, isOutput=True)

    with tile.TileContext(nc) as tc, ExitStack() as ctx:
        const_pool = ctx.enter_context(tc.tile_pool(name="const", bufs=1))
        tmp_pool = ctx.enter_context(tc.tile_pool(name="tmp", bufs=2))
        work_pool = ctx.enter_context(tc.tile_pool(name="work", bufs=2))
        idx_pool = ctx.enter_context(tc.tile_pool(name="idx", bufs=2))
        z_pool = ctx.enter_context(tc.tile_pool(name="z", bufs=3))
        psum_pool = ctx.enter_context(
            tc.tile_pool(name="psum", bufs=psum_bufs, space="PSUM")
        )
        mm_pool = ctx.enter_context(tc.tile_pool(name="mmbuf", bufs=4))

        def load_tiles(param, width, dtype, tag, split=1):
            r = param.ap().rearrange("(t p) m -> t p m", p=P)
            out = []
            cw = width // split
            for t in range(ndt):
                tl = const_pool.tile([P, width], dtype, tag=f"{tag}{t}")
                for s in range(split):
                    cs = slice(s * cw, (s + 1) * cw)
                    nc.sync.dma_start(tl[:, cs], r[t][:, cs])
                out.append(tl)
            return out

        if mode == "f32":
            ct_tiles = load_tiles(ct, k, f32, "ct")
            x2t_tiles = load_tiles(x2t, n, f32, "x2t")
            prime_tiles = ct_tiles + x2t_tiles
        else:
            cth_tiles = load_tiles(ct_hi, k, f16, "cth")
            ctl_tiles = load_tiles(ct_lo, k, f16, "ctl")
            xh_tiles = load_tiles(x2t_hi, n, f16, "xh")
            xl_tiles = load_tiles(x2t_lo, n, f16, "xl")
            prime_tiles = cth_tiles + ctl_tiles + xh_tiles + xl_tiles
        cbs_tile = const_pool.tile([P, k], f32, tag="cbs")
        nc.sync.dma_start(cbs_tile[:], cbs_rep.ap())
        xsq_tile = const_pool.tile([P, nt], f32, tag="xsq")
        nc.sync.dma_start(xsq_tile[:], xsq.ap().rearrange("t p -> p t"))

        # Prime each engine's view of the const-tile DMAs one sem at a time so
        # hot-loop instructions keep a single sync wait (HW structs hold one;
        # extras cost EVSEM instructions).
        prime_pool = ctx.enter_context(
            tc.tile_pool(name="prime_psum", bufs=1, space="PSUM")
        )
        prime_ps = prime_pool.tile([1, 1], f32, tag="prime")
        for t in prime_tiles:
            nc.tensor.matmul(
                out=prime_ps[:], lhsT=t[:, 0:1], rhs=t[:, 0:1],
                start=True, stop=True,
            )
        prime_dve = const_pool.tile([P, 2], f32, tag="prime_dve")
        nc.vector.tensor_copy(prime_dve[:, 0:1], cbs_tile[:, 0:1])
        nc.vector.tensor_copy(prime_dve[:, 1:2], xsq_tile[:, 0:1])
        prime_act = const_pool.tile([P, 2], f32, tag="prime_act")
        nc.scalar.activation(
            prime_act[:, 0:1], cbs_tile[:, 0:1],
            mybir.ActivationFunctionType.Identity, bias=xsq_tile[:, 0:1],
        )

        for rt in range(nt):
            rows = slice(rt * P, (rt + 1) * P)
            # tmp = fl(cb_sqr + x_sqr): one fp32 add on the scalar engine,
            # rounding-identical to the reference's broadcast add.
            tmp = tmp_pool.tile([P, k], f32, tag="tmp")
            nc.scalar.activation(
                tmp[:], cbs_tile[:],
                mybir.ActivationFunctionType.Identity,
                bias=xsq_tile[:, rt : rt + 1],
            )
            nd = work_pool.tile([P, k], f32, tag="nd")

            for kc in range(nkc):
                kslice = slice(kc * kc_width, (kc + 1) * kc_width)
                ps = psum_pool.tile([P, kc_width], f32, tag="mm")
                if mode == "f32":
                    for t in range(ndt):
                        nc.tensor.matmul(
                            out=ps[:],
                            lhsT=x2t_tiles[t][:, rows],
                            rhs=ct_tiles[t][:, kslice],
                            start=(t == 0),
                            stop=(t == ndt - 1),
                        )
                else:
                    i = 0
                    for t in range(ndt):
                        for lhs, rhs in (
                            (xh_tiles[t], cth_tiles[t]),
                            (xl_tiles[t], cth_tiles[t]),
                            (xh_tiles[t], ctl_tiles[t]),
                        ):
                            nc.tensor.matmul(
                                out=ps[:],
                                lhsT=lhs[:, rows],
                                rhs=rhs[:, kslice],
                                start=(i == 0),
                                stop=(i == 3 * ndt - 1),
                            )
                            i += 1
                # nd = S - fl(cb_sqr + x_sqr)
                if offload:
                    mmb = mm_pool.tile([P, kc_width], f32, tag="mmb")
                    nc.scalar.activation(
                        mmb[:], ps[:], mybir.ActivationFunctionType.Identity,
                    )
                    nc.gpsimd.tensor_tensor(
                        out=nd[:, kslice], in0=mmb[:], in1=tmp[:, kslice],
                        op=mybir.AluOpType.subtract,
                    )
                else:
                    nc.vector.tensor_tensor(
                        out=nd[:, kslice], in0=ps[:], in1=tmp[:, kslice],
                        op=mybir.AluOpType.subtract,
                    )

            mx8 = idx_pool.tile([P, 8], f32, tag="mx8")
            mi8 = idx_pool.tile([P, 8], mybir.dt.uint32, tag="mi8")
            if use_reduce_max:
                nc.gpsimd.memset(mx8[:, 1:8], -3.0e38)
                nc.vector.reduce_max(mx8[:, 0:1], nd[:], axis=mybir.AxisListType.X)
            else:
                nc.vector.max(mx8[:], nd[:])
            nc.vector.max_index(mi8[:], mx8[:], nd[:])
            # mi8's only readers are same-engine copies, so max_index never
            # waits on DMA slot reuse; each copy carries one WAR wait.
            idxg = idx_pool.tile([P, 1], mybir.dt.uint32, tag="idxg")
            idxs = idx_pool.tile([P, 1], mybir.dt.uint32, tag="idxs")
            nc.vector.tensor_copy(prime_dve[:, 1:2], mi8[:, 0:1].bitcast(f32))
            nc.vector.tensor_copy(idxg[:], mi8[:, 0:1])
            nc.vector.tensor_copy(idxs[:], mi8[:, 0:1])
            nc.sync.dma_start(idx_out.ap()[rows, :], idxs[:])

            ztile = z_pool.tile([P, d], f32, tag="z")
            nc.gpsimd.indirect_dma_start(
                out=ztile[:],
                out_offset=None,
                in_=cb.ap(),
                in_offset=bass.IndirectOffsetOnAxis(ap=idxg[:], axis=0),
            )
            nc.sync.dma_start(z_out.ap()[rows, :], ztile[:])

    nc.finalize()
    return nc


_CACHED = {}
LAST_RESULTS = None


def _get_nc(shape_key):
    if shape_key not in _CACHED:
        _CACHED[shape_key] = build_bass()
    return _CACHED[shape_key]


def make_in_maps(z_e_x, codebook, mode=None):
    mode = mode or MM_MODE
    ct32 = np.ascontiguousarray(codebook.T)
    cbs = np.sum(np.square(codebook), axis=1, dtype=np.float32)
    if mode == "f32":
        shared = {"ct": ct32, "cb": codebook}
        scale = np.float32(1.0)
    else:
        cst = ct32 * np.float32(4096.0)  # exact power-of-two scale
        ch = cst.astype(np.float16)
        cl = (cst - ch.astype(np.float32)).astype(np.float16)
        shared = {"ct_hi": ch, "ct_lo": cl, "cb": codebook}
        scale = np.float32(4096.0)
    shared["cbs_rep"] = np.ascontiguousarray(
        np.broadcast_to((cbs * scale)[None, :], (P, K))
    )

    in_maps = []
    for c in range(N_CORES):
        xc = z_e_x[c]  # [S, D]
        x2t = np.ascontiguousarray((xc * np.float32(2.0)).T)  # exact x2 scale
        xsq = (np.sum(np.square(xc), axis=1, dtype=np.float32) * scale).reshape(
            S // P, P
        )
        m = dict(shared)
        if mode == "f32":
            m["x2t"] = x2t
        else:
            xh = x2t.astype(np.float16)
            xl = (x2t - xh.astype(np.float32)).astype(np.float16)
            m["x2t_hi"] = xh
            m["x2t_lo"] = xl
        m["xsq"] = np.ascontiguousarray(xsq)
        in_maps.append(m)
    return in_maps


def kernel(z_e_x: np.ndarray, codebook: np.ndarray):
    z_e_x = np.ascontiguousarray(z_e_x, dtype=np.float32)
    codebook = np.ascontiguousarray(codebook, dtype=np.float32)
    assert z_e_x.shape == (B, S, D) and codebook.shape == (K, D)

    nc = _get_nc((B, S, D, K, MM_MODE))
    in_maps = make_in_maps(z_e_x, codebook)

    trace = bool(os.environ.get("VQ_TRACE"))
    kres = run_bass_kernel_spmd(
        nc,
        in_maps,
        core_ids=list(range(N_CORES)),
        trace=trace,
        **({"trace_cores": [0]} if trace else {}),
    )
    global LAST_RESULTS
    LAST_RESULTS = kres
    results = kres.results

    z = np.stack([r["z_out"] for r in results]).reshape(B, S, D)
    idx = (
        np.stack([r["idx_out"] for r in results])
        .reshape(B, S)
        .astype(np.int32)
    )
    return (z, z, idx)
